# revision 20
# baseline (speedup 1.0000x reference)
"""Trainium2 Bass kernel for nn_DictNet loss (8-core SPMD), fp16 edition.

Math restructuring
------------------
reference(D, x, C, mask, y, groups) decomposes as:

  Cn    = C / ||C||                      (tiny, host)
  L     = einsum('nmk,k->nm', D, Cn)     (memory-bound: D dominates traffic)
  y_hat = x - L @ x
  d     = pairwise distance matrix of y_hat rows   [N, N]
  loss  = sparsity(Cn) + sum_c u_c d u_c^T - (1/(S^2*beta)) * sum_g h_g d h_g^T

where u[c, n] = (mask_n & y_n == c) / cnt_c  (hl2 per-class weights) and
h[g, n] = multiplicity of node n in groups[g]  (hl1 group histograms).
Both weight matrices are tiny and precomputed on the host; the device never
touches an index. Per-core partial sums are combined on the host.

Device-side layout/precision choices:
- D is shipped fp16 in planar-k layout [K, R, N] so the DVE k-contraction
  (AXPY) reads packed 16-bit planes (2x DVE mode) and DMA traffic halves.
- Phase A is rc-major with x resident in SBUF: y psum needs only 2 banks.
- The distance tiles are computed as one PSUM accumulation:
      d2 = (-2*y_own)^T @ y_col + ones^T @ sn_col      (augmented matmul)
      d  = sqrt(d2 + sn_own)                           (activation bias)
  so no separate sn broadcast / add / max passes are needed. The j=0
  diagonal is zeroed in PSUM (and d after sqrt) with a device-built
  inverse-identity tile.
- Everything large is fp16 (y_hat, AllGather buffers, weights); distance
  accumulation stays in fp32 PSUM.

Sharding: D rows (node axis) split across 8 cores; y_hat rows AllGathered
(transposed) so every core can form distance tiles for its own rows.
Symmetry: d and the weights are symmetric, so each core only processes
JBLK = CORES/2 + 1 rotated column blocks (global block (pid+j) % CORES);
off-diagonal blocks are double-counted via host-scaled weights, and the
j = CORES/2 block is zero-weighted on the upper half of the cores.

With pipe=True (reps > 1), phase D of rep r-1 is emitted after rep r's
AllGather, so rep r's D-tile DMAs overlap rep r-1's distance phase
(steady-state throughput measurement).
"""

import math

import numpy as np

import concourse.bass as bass
import concourse.mybir as mybir
import concourse.tile as tile
from concourse import bacc
from concourse.bass_utils import run_bass_kernel_spmd
from concourse.masks import make_identity

FP32 = mybir.dt.float32
FP16 = mybir.dt.float16
FP8 = mybir.dt.float8e4
AF = mybir.ActivationFunctionType
OP = mybir.AluOpType

# d8: ship D as fp8e4 scaled by 32 (cn identities scaled by 16; the 1/512
# descale is folded into the phase-B subtraction).
FULL_CFG = dict(N=4096, F=512, K=11, G=128, NCLS=7, CORES=8, d8=False)
D_SCALE = 32.0
CN_SCALE = 16.0


def _derived(cfg):
    N, F, K, G, NCLS, CORES = (
        cfg["N"], cfg["F"], cfg["K"], cfg["G"], cfg["NCLS"], cfg["CORES"])
    R = N // CORES              # rows per core
    assert R % 128 == 0 and N % 512 == 0 and F % 128 == 0
    NRC = R // 128              # 128-row chunks per core
    NMC = N // 128              # 128-col chunks (m axis)
    MGRP = 1024                 # m columns per D tile
    NGRP = N // MGRP            # D tile groups along m
    SUBS = MGRP // 128
    NFC = F // 128              # feature chunks
    JBLK = CORES // 2 + 1       # rotated col blocks each core processes
    return dict(N=N, F=F, K=K, G=G, NCLS=NCLS, CORES=CORES, R=R, NRC=NRC,
                NMC=NMC, MGRP=MGRP, NGRP=NGRP, SUBS=SUBS, NFC=NFC, JBLK=JBLK)


def build(cfg, reps=1, stage="full", pipe=True):
    """Build the SPMD kernel (one NEFF, runs on all cores).

    reps > 1 repeats the whole computation serially (timing probe).
    stage: "dma" = D loads only, "axpy" = + AXPY, "A" = phases A+B,
    "sim" = full with faked collective (for TimelineSim), "full" = real.
    """
    c = _derived(cfg)
    N, F, K, G, NCLS = c["N"], c["F"], c["K"], c["G"], c["NCLS"]
    CORES, R, NRC, NMC = c["CORES"], c["R"], c["NRC"], c["NMC"]
    MGRP, NGRP, SUBS, NFC, JBLK = (
        c["MGRP"], c["NGRP"], c["SUBS"], c["NFC"], c["JBLK"])

    nc = bacc.Bacc("TRN2", target_bir_lowering=False, debug=False,
                   num_devices=CORES)

    # ---- I/O ----
    d8 = bool(cfg.get("d8"))
    DDT = FP8 if d8 else FP16
    descale = -1.0 / (D_SCALE * CN_SCALE) if d8 else -1.0
    Dsh = nc.dram_tensor("Dsh", [K, R, N], DDT, kind="ExternalInput")
    x_in = nc.dram_tensor("x_in", [N, F], FP16, kind="ExternalInput")
    x_own = nc.dram_tensor("x_own", [R, F], FP16, kind="ExternalInput")
    cnb_in = nc.dram_tensor("cnb", [128, K], FP32, kind="ExternalInput")
    uT_in = nc.dram_tensor("uT_sh", [R, NCLS], FP16, kind="ExternalInput")
    hT_in = nc.dram_tensor("hT_sh", [R, G], FP16, kind="ExternalInput")
    u_in = nc.dram_tensor("u_rot", [NCLS, JBLK, R], FP16, kind="ExternalInput")
    h_in = nc.dram_tensor("h_rot", [G, JBLK, R], FP16, kind="ExternalInput")
    out_u = nc.dram_tensor("out_u", [NCLS, JBLK], FP32, kind="ExternalOutput")
    out_h = nc.dram_tensor("out_h", [G, JBLK], FP32, kind="ExternalOutput")

    # collective bounce buffers (parity-doubled for cross-rep pipelining):
    # rows 0..F-1 = y_hat^T (own cols) fp16, row F = sn fp16
    agin = [nc.dram_tensor(f"agin{p}", [F + 1, R], FP16) for p in range(2)]
    agout = [nc.dram_tensor(f"agout{p}", [CORES, F + 1, R], FP16,
                            addr_space="Shared") for p in range(2)]

    with tile.TileContext(nc) as tc:
      with (
          tc.tile_pool(name="persist", bufs=1) as pp,
          tc.tile_pool(name="dA", bufs=6) as dpool,
          tc.tile_pool(name="xrA", bufs=1) as xr_pool,
          tc.tile_pool(name="cnA", bufs=2) as cn_pool,
          tc.tile_pool(name="lgA", bufs=3) as lg_pool,
          tc.tile_pool(name="tmpA", bufs=3) as tmp_pool,
          tc.tile_pool(name="lgpA", bufs=2) as lgp_pool,
          tc.tile_pool(name="ltA", bufs=2) as ltsb_pool,
          tc.tile_pool(name="xoB", bufs=2) as xo_pool,
          tc.tile_pool(name="sqB", bufs=2) as sq_pool,
          tc.tile_pool(name="wD", bufs=1) as w_pool,
          tc.tile_pool(name="yrD", bufs=1) as yr_pool,
          tc.tile_pool(name="snD", bufs=2) as sn_pool,
          tc.tile_pool(name="dD", bufs=4) as dd_pool,
          tc.tile_pool(name="ttD", bufs=2) as tt_pool,
          tc.tile_pool(name="psY", bufs=2, space="PSUM") as psY,
          tc.tile_pool(name="psTr", bufs=2, space="PSUM") as psTr,
          tc.tile_pool(name="psG", bufs=2, space="PSUM") as psG,
          tc.tile_pool(name="psV", bufs=1, space="PSUM") as psV,
      ):
        ident = pp.tile([128, 128], FP16)
        make_identity(nc, ident[:])
        invident = pp.tile([128, 128], FP16)
        nc.gpsimd.memset(invident[:], 1.0)
        nc.gpsimd.affine_select(
            out=invident[:], in_=invident[:],
            compare_op=OP.not_equal, fill=0.0, base=0,
            pattern=[[-1, 128]], channel_multiplier=1)
        onesrow = pp.tile([1, 128], FP16)
        nc.vector.memset(onesrow[:], 1.0)

        # rotation registers: (pid + j) % CORES for j >= 1 (hoisted)
        sp_eng = nc.engines[mybir.EngineType.SP]
        pid = sp_eng.partition_id()
        rot = []
        for j in range(1, JBLK):
            rj = sp_eng.alloc_register(f"rot{j}")
            sp_eng.reg_alu(rj, pid, j, OP.add)
            sp_eng.reg_alu(rj, rj, CORES, OP.mod)
            rot.append(bass.make_scalar_value(rj, min_val=0,
                                              max_val=CORES - 1))

        # parity-doubled persistent state
        y_sb = [pp.tile([128, F], FP16, name=f"y_sb{rc}")
                for rc in range(NRC)]
        sn_own = [[pp.tile([128, 1], FP32, name=f"sn_own{p}_{rc}")
                   for rc in range(NRC)] for p in range(2)]
        sn16 = [pp.tile([128, 1], FP16, name=f"sn16_{rc}")
                for rc in range(NRC)]
        yT_own = [[pp.tile([128, R], FP16, name=f"yT_own{p}_{fc}")
                   for fc in range(NFC)] for p in range(2)]
        yTm2 = [[pp.tile([128, R], FP16, name=f"yTm2_{p}_{fc}")
                 for fc in range(NFC)] for p in range(2)]
        acc_u = pp.tile([NCLS, JBLK], FP32)
        acc_h = pp.tile([G, JBLK], FP32)

        def phase_A(rep, par):
            """L^T = sum_k cn_k D_k^T via PE (matmul against cn_k-scaled
            identity, PSUM-accumulated over k); y = x - L @ x; sn; y^T."""
            cnb = cn_pool.tile([128, K], FP32, tag="cnb")
            nc.sync.dma_start(cnb[:], cnb_in[:])
            xres = xr_pool.tile([128, NMC, F], FP16, tag="xres")
            nc.sync.dma_start(
                xres[:], x_in[:].rearrange("(mc p) f -> p mc f", p=128))
            # DVE handles planes 0..KD-1 (4x-mode tensor_scalar + 2x adds),
            # Pool handles planes KD..K-1 (fused scalar_tensor_tensor), one
            # DVE add merges. Plane loads are whole [128, N] rows: one
            # contiguous 8KB run per partition per DMA.
            KD = 7
            dve_ks = list(range(KD))
            pool_ks = list(range(KD, K))
            # interleave so Pool's first plane arrives early
            load_order = []
            di, pi = 0, 0
            for i in range(K):
                if i % 2 == 1 and pi < len(pool_ks):
                    load_order.append(pool_ks[pi]); pi += 1
                elif di < len(dve_ks):
                    load_order.append(dve_ks[di]); di += 1
                else:
                    load_order.append(pool_ks[pi]); pi += 1
            for rc in range(NRC):
                ypsum = psY.tile([128, F], FP32, tag="yp",
                                 name="ypsum")
                planes = {}
                for k in load_order:
                    pk = dpool.tile([128, N], DDT, tag="D", name="pk")
                    nc.sync.dma_start(
                        pk[:], Dsh[k, rc * 128:(rc + 1) * 128, :])
                    planes[k] = pk
                if stage == "dma":
                    continue
                lg = lg_pool.tile([128, N], FP16, tag="L", name="lg")
                nc.vector.tensor_scalar_mul(lg[:], planes[0][:],
                                            cnb[:, 0:1])
                for k in range(1, KD):
                    tmp = tmp_pool.tile([128, N], FP16, tag="t",
                                        name="tmp")
                    nc.vector.tensor_scalar_mul(tmp[:], planes[k][:],
                                                cnb[:, k:k + 1])
                    nc.vector.tensor_tensor(out=lg[:], in0=lg[:],
                                            in1=tmp[:], op=OP.add)
                for k in range(KD, K):
                    tmp = tmp_pool.tile([128, N], FP16, tag="t",
                                        name="tmp")
                    nc.vector.tensor_scalar_mul(tmp[:], planes[k][:],
                                                cnb[:, k:k + 1])
                    nc.vector.tensor_tensor(out=lg[:], in0=lg[:],
                                            in1=tmp[:], op=OP.add)
                if stage == "axpy":
                    continue
                # PE transposes 512 L cols per psum tile, then matmul
                for half in range(2 * NGRP):
                    ltp = psTr.tile([128, 512], FP16, tag="tr",
                                    name="ltp")
                    for q in range(4):
                        sub = half * 4 + q
                        nc.tensor.transpose(
                            ltp[:, q * 128:(q + 1) * 128],
                            lg[:, sub * 128:(sub + 1) * 128],
                            ident[:])
                    lts = ltsb_pool.tile([128, 512], FP16, tag="LTS")
                    nc.scalar.copy(lts[:], ltp[:])
                    for q in range(4):
                        mc = half * 4 + q
                        nc.tensor.matmul(
                            ypsum[:],
                            lhsT=lts[:, q * 128:(q + 1) * 128],
                            rhs=xres[:, mc, :],
                            start=(mc == 0), stop=(mc == NMC - 1))
                if stage in ("dma", "axpy"):
                    continue
                # ---- phase B for this rc ----
                xo = xo_pool.tile([128, F], FP16, tag="xo")
                nc.sync.dma_start(xo[:], x_own[rc * 128:(rc + 1) * 128, :])
                nc.vector.scalar_tensor_tensor(
                    y_sb[rc][:], ypsum[:], descale, xo[:], OP.mult, OP.add)
                sq = sq_pool.tile([128, F], FP16, tag="sq")
                nc.scalar.activation(
                    sq[:], y_sb[rc][:], AF.Square,
                    accum_out=sn_own[par][rc][:])
                nc.scalar.copy(sn16[rc][:], sn_own[par][rc][:])
                # transpose y rows -> yT column block rc
                ytp = psTr.tile([128, 512], FP16, tag="tr",
                                name="ytp")
                for fc in range(NFC):
                    nc.tensor.transpose(
                        ytp[:, fc * 128:(fc + 1) * 128],
                        y_sb[rc][:, fc * 128:(fc + 1) * 128],
                        ident[:])
                for fc in range(NFC):
                    nc.scalar.copy(
                        yT_own[par][fc][:, rc * 128:(rc + 1) * 128],
                        ytp[:, fc * 128:(fc + 1) * 128])
                    nc.vector.tensor_scalar_mul(
                        yTm2[par][fc][:, rc * 128:(rc + 1) * 128],
                        ytp[:, fc * 128:(fc + 1) * 128], -2.0)
                    nc.sync.dma_start(
                        agin[par][fc * 128:(fc + 1) * 128,
                                  rc * 128:(rc + 1) * 128],
                        yT_own[par][fc][:, rc * 128:(rc + 1) * 128])
                nc.sync.dma_start(
                    agin[par][F:F + 1, rc * 128:(rc + 1) * 128]
                    .rearrange("one p -> p one"),
                    sn16[rc][:])

        def phase_AG(rep, par):
            if stage == "sim":
                for r in range(CORES):
                    nc.sync.dma_start(agout[par][r], agin[par][:])
            else:
                nc.gpsimd.collective_compute(
                    "AllGather", OP.bypass,
                    replica_groups=[list(range(CORES))],
                    ins=[agin[par][:]], outs=[agout[par][0:CORES]])

        def phase_D(rep, par):
            """distance tiles + weighted sums -> out_u/out_h."""
            yT_rot = [yr_pool.tile([128, JBLK - 1, R], FP16, tag=f"yTr{fc}",
                                   name=f"yT_rot{fc}")
                      for fc in range(NFC)]
            for fc in range(NFC):
                for j in range(1, JBLK):
                    nc.sync.dma_start(
                        yT_rot[fc][:, j - 1, :],
                        agout[par][bass.ds(rot[j - 1], 1),
                                   fc * 128:(fc + 1) * 128, :]
                        .rearrange("r f n -> f (r n)"))
            sn_rot = sn_pool.tile([1, JBLK - 1, R], FP16, tag="snr",
                                  name="sn_rot")
            for j in range(1, JBLK):
                nc.sync.dma_start(
                    sn_rot[:, j - 1, :],
                    agout[par][bass.ds(rot[j - 1], 1), F:F + 1, :]
                    .rearrange("r one n -> one (r n)"))
            sn_loc = sn_pool.tile([1, R], FP16, tag="snl", name="sn_loc")
            nc.sync.dma_start(sn_loc[:], agin[par][F:F + 1, :])

            uT_sb = w_pool.tile([128, NRC, NCLS], FP16, tag="uT")
            nc.sync.dma_start(
                uT_sb[:], uT_in[:].rearrange("(rc p) c -> p rc c", p=128))
            hT_sb = w_pool.tile([128, NRC, G], FP16, tag="hT")
            nc.sync.dma_start(
                hT_sb[:], hT_in[:].rearrange("(rc p) g -> p rc g", p=128))
            u_sb = w_pool.tile([NCLS, JBLK, R], FP16, tag="u")
            nc.sync.dma_start(u_sb[:], u_in[:])
            h_sb = w_pool.tile([G, JBLK, R], FP16, tag="h")
            nc.sync.dma_start(h_sb[:], h_in[:])

            vu = vh = None
            pending = None  # (j, rc, d_tile) awaiting V matmuls

            def flush_pending():
                nonlocal pending
                if pending is None:
                    return
                pj, prc, pdt = pending
                nc.tensor.matmul(
                    vu[:], lhsT=uT_sb[:, prc, :], rhs=pdt[:],
                    start=(prc == 0), stop=(prc == NRC - 1))
                nc.tensor.matmul(
                    vh[:], lhsT=hT_sb[:, prc, :], rhs=pdt[:],
                    start=(prc == 0), stop=(prc == NRC - 1))
                pending = None
                if prc == NRC - 1:
                    su = tt_pool.tile([NCLS, R], FP16, tag="su",
                                      name="su")
                    nc.vector.tensor_tensor(
                        out=su[:], in0=vu[:], in1=u_sb[:, pj, :],
                        op=OP.mult)
                    nc.vector.reduce_sum(
                        acc_u[:, pj:pj + 1], su[:],
                        axis=mybir.AxisListType.X)
                    sh = tt_pool.tile([G, R], FP16, tag="sh",
                                      name="sh")
                    nc.vector.tensor_tensor(
                        out=sh[:], in0=vh[:], in1=h_sb[:, pj, :],
                        op=OP.mult)
                    nc.vector.reduce_sum(
                        acc_h[:, pj:pj + 1], sh[:],
                        axis=mybir.AxisListType.X)

            for j in range(JBLK):
                for rc in range(NRC):
                    if rc == 0:
                        new_vu = psV.tile([NCLS, R], FP32, tag="vu",
                                          name="vu")
                        new_vh = psV.tile([G, R], FP32, tag="vh",
                                          name="vh")
                    gram = psG.tile([128, R], FP32, tag="g",
                                    name="gram")
                    for fc in range(NFC):
                        rhs = (yT_own[par][fc][:] if j == 0
                               else yT_rot[fc][:, j - 1, :])
                        nc.tensor.matmul(
                            gram[:],
                            lhsT=yTm2[par][fc][:, rc * 128:(rc + 1) * 128],
                            rhs=rhs,
                            start=(fc == 0), stop=False)
                    snsrc = sn_loc[:] if j == 0 else sn_rot[:, j - 1, :]
                    nc.tensor.matmul(
                        gram[:], lhsT=onesrow[:], rhs=snsrc,
                        start=False, stop=True)
                    flush_pending()
                    if rc == 0:
                        vu, vh = new_vu, new_vh
                    if j == 0:
                        nc.vector.tensor_tensor(
                            out=gram[:, rc * 128:(rc + 1) * 128],
                            in0=gram[:, rc * 128:(rc + 1) * 128],
                            in1=invident[:], op=OP.mult)
                    dt = dd_pool.tile([128, R], FP16, tag="d")
                    nc.scalar.activation(dt[:], gram[:], AF.Sqrt,
                                         bias=sn_own[par][rc][:])
                    if j == 0:
                        nc.vector.tensor_tensor(
                            out=dt[:, rc * 128:(rc + 1) * 128],
                            in0=dt[:, rc * 128:(rc + 1) * 128],
                            in1=invident[:], op=OP.mult)
                    pending = (j, rc, dt)
            flush_pending()
            nc.sync.dma_start(out_u[:], acc_u[:])
            nc.sync.dma_start(out_h[:], acc_h[:])

        if stage in ("dma", "axpy", "A"):
            for rep in range(reps):
                phase_A(rep, rep % 2)
            nc.vector.memset(acc_u[:], 0.0)
            nc.vector.memset(acc_h[:], 0.0)
            nc.sync.dma_start(out_u[:], acc_u[:])
            nc.sync.dma_start(out_h[:], acc_h[:])
        elif pipe:
            for rep in range(reps):
                par = rep % 2
                phase_A(rep, par)
                phase_AG(rep, par)
                if rep > 0:
                    phase_D(rep - 1, (rep - 1) % 2)
            phase_D(reps - 1, (reps - 1) % 2)
        else:
            for rep in range(reps):
                par = rep % 2
                phase_A(rep, par)
                phase_AG(rep, par)
                phase_D(rep, par)

    nc.compile()
    return nc


def host_prep(cfg, D, x, C, mask, y, groups):
    """Host-side input prep: normalize C, build weight matrices, shard."""
    c = _derived(cfg)
    N, K, G, NCLS, CORES, R = c["N"], c["K"], c["G"], c["NCLS"], c["CORES"], c["R"]
    JBLK = c["JBLK"]

    C32 = np.asarray(C, np.float32)
    cn = (C32 / np.linalg.norm(C32, axis=0, keepdims=True)).astype(np.float32)
    dim = np.float32(math.sqrt(K))
    nrm = np.linalg.norm(cn, axis=0).astype(np.float32)
    sparsity = float(np.mean((dim - np.abs(cn).sum(0) / nrm) / (dim - 1.0)))

    mask_b = np.asarray(mask, bool)
    y_i = np.asarray(y, np.int64)
    cnt = np.zeros(NCLS, np.int64)
    np.add.at(cnt, y_i[mask_b], 1)
    u = np.zeros((NCLS, N), np.float32)
    sel = mask_b & (cnt[y_i] > 0)
    u[y_i[sel], np.nonzero(sel)[0]] = 1.0 / cnt[y_i[sel]]

    g_i = np.asarray(groups, np.int64)
    H = np.zeros((G, N), np.float32)
    np.add.at(H, (np.repeat(np.arange(G), g_i.shape[1]), g_i.ravel()), 1.0)

    cn_s = cn * (np.float32(CN_SCALE) if cfg.get("d8") else np.float32(1.0))
    cnb = np.tile(cn_s.ravel()[None, :], (128, 1)).astype(np.float32)
    uT = np.ascontiguousarray(u.T.astype(np.float16))
    hT = np.ascontiguousarray(H.T.astype(np.float16))
    x16 = np.ascontiguousarray(np.asarray(x, np.float32).astype(np.float16))
    D32 = np.asarray(D, np.float32)

    in_maps = []
    for ci in range(CORES):
        sl = slice(ci * R, (ci + 1) * R)
        # rotated, symmetry-scaled weight slices: j -> global block (ci+j)%CORES
        u_rot = np.zeros((NCLS, JBLK, R), np.float16)
        h_rot = np.zeros((G, JBLK, R), np.float16)
        for j in range(JBLK):
            gb = (ci + j) % CORES
            scale = 1.0 if j == 0 else 2.0
            if j == CORES // 2 and ci >= CORES // 2:
                continue  # pair already handled by core ci - CORES//2
            u_rot[:, j, :] = u[:, gb * R:(gb + 1) * R] * scale
            h_rot[:, j, :] = H[:, gb * R:(gb + 1) * R] * scale
        # planar-k D shard: [K, R, N]
        if cfg.get("d8"):
            import ml_dtypes
            Dk = np.ascontiguousarray(
                (D32[sl].transpose(2, 0, 1) * np.float32(D_SCALE))
                .astype(ml_dtypes.float8_e4m3))
        else:
            Dk = np.ascontiguousarray(
                D32[sl].transpose(2, 0, 1).astype(np.float16))
        in_maps.append({
            "Dsh": Dk,
            "x_in": x16,
            "x_own": np.ascontiguousarray(x16[sl]),
            "cnb": cnb,
            "uT_sh": np.ascontiguousarray(uT[sl]),
            "hT_sh": np.ascontiguousarray(hT[sl]),
            "u_rot": u_rot,
            "h_rot": h_rot,
        })
    return in_maps, sparsity


def combine(cfg, results, sparsity, group_size):
    """loss = sparsity + hl2 + hl1/beta, from per-core partial sums."""
    beta = np.float64(cfg["G"]) / np.float64(cfg["NCLS"])
    hl2 = np.float64(0.0)
    s1 = np.float64(0.0)
    for r in results:
        hl2 += r["out_u"].astype(np.float64).sum()
        s1 += r["out_h"].astype(np.float64).sum()
    hl1 = -s1 / np.float64(group_size * group_size)
    total = np.float64(sparsity) + hl2 + hl1 / beta
    return np.float32(total)


_BUILD_CACHE = {}


def _get_nc(key, cfg):
    if key not in _BUILD_CACHE:
        _BUILD_CACHE[key] = build(cfg)
    return _BUILD_CACHE[key]


def kernel(D, x, C, mask, y, groups):
    cfg = dict(FULL_CFG)
    key = "full8" if cfg.get("d8") else "full"
    in_maps, sparsity = host_prep(cfg, D, x, C, mask, y, groups)
    nc = _get_nc(key, cfg)
    res = run_bass_kernel_spmd(
        nc, in_maps, core_ids=list(range(cfg["CORES"])), trace=False)
    return combine(cfg, res.results, sparsity, np.asarray(groups).shape[1])
